# revision 27
# baseline (speedup 1.0000x reference)
"""ALiBi attention (B=2, S=2048, D=1024, H=16) on 8 TRN2 NeuronCores.

Sharding: core c handles batch b = c//4 and query slice qs = (c%4)*512.
Key insight: the reference applies bias slope_h*(k-q) with NO causal mask and
slopes in [0.52, 1.0], so softmax mass sits entirely on the last ~60 keys.
Keeping only the last KW=128 keys gives max attention-weight error ~1e-29.
Furthermore exp(qk*scale + slope*(k-q) - rowmax(q)) with rowmax ~= slope*(S-1-q)
reduces to exp(qk*scale + slope*(k-S+1)): the bias is purely a function of k,
i.e. a per-partition constant in the [k, q] layout -> single fused ACT op.

Per core:
  QT[ch,q]   = Wq^T x^T        (8 ch-tiles x 8 d-tiles, N=512)
  KT[ch,k]   = Wk^T xk^T       (8 x 8, N=128)
  V[k,ch]    = xk Wv           (2 blocks x 8 d-tiles, N=512) + ones col per head
  PT_h[k,q]  = exp(scale*KT_h^T QT_h + cbias_h)      (ACT, per-partition bias)
  denom      = ones^T PT_h     -> reciprocal -> PE outer-product broadcast
  outT_h     = V_h^T PT_h, normalized by denom       (col-tiled into head pairs)
  out[q,d]   = outT^T Wo       (+ bo on host)
No collectives: cores are fully independent; host concatenates query slices.
"""

import numpy as np
import ml_dtypes

D = 1024
H = 16
HD = 64
B = 2
S = 2048
QS = 512          # queries per core
KW = 128          # key window (last KW keys carry all softmax mass)
K0 = S - KW
NT = 8            # 128-wide tiles over D
P = 128
SCALE = HD ** -0.5
N_CORES = 8

_CACHE = {}


def _build():
    import concourse.bacc as bacc
    import concourse.mybir as mybir
    import concourse.tile as tile
    from concourse.masks import make_identity

    BF = mybir.dt.bfloat16
    F32 = mybir.dt.float32
    AF = mybir.ActivationFunctionType

    nc = bacc.Bacc("TRN2", target_bir_lowering=False, debug=False, num_devices=N_CORES)

    xTq = nc.dram_tensor("xTq", [D, QS], BF, kind="ExternalInput").ap()
    xTk = nc.dram_tensor("xTk", [D, KW], BF, kind="ExternalInput").ap()
    Wq = nc.dram_tensor("Wq", [D, D], BF, kind="ExternalInput").ap()
    Wk = nc.dram_tensor("Wk", [D, D], BF, kind="ExternalInput").ap()
    Wv = nc.dram_tensor("Wv", [D, D], BF, kind="ExternalInput").ap()
    Wo = nc.dram_tensor("Wo", [D, D], BF, kind="ExternalInput").ap()
    bq = nc.dram_tensor("bq", [P, NT], F32, kind="ExternalInput").ap()
    bk = nc.dram_tensor("bk", [P, NT], F32, kind="ExternalInput").ap()
    bv = nc.dram_tensor("bv", [1, D], BF, kind="ExternalInput").ap()
    cb = nc.dram_tensor("cbias", [KW, H], F32, kind="ExternalInput").ap()
    out = nc.dram_tensor("out", [QS, D], F32, kind="ExternalOutput").ap()

    with tile.TileContext(nc) as tc:
        with (
            tc.tile_pool(name="wpool", bufs=1) as wp,
            tc.tile_pool(name="dpool", bufs=1) as dp,
            tc.tile_pool(name="flow", bufs=3) as fp,
            tc.tile_pool(name="pacc", bufs=2, space="PSUM") as pacc,
            tc.tile_pool(name="patt", bufs=2, space="PSUM") as patt,
            tc.tile_pool(name="psc", bufs=2, space="PSUM") as psc,
            tc.tile_pool(name="psml", bufs=2, space="PSUM") as psml,
        ):
            # ---- load inputs. Per-d-tile chunks (256KB) alternating across the
            # two HWDGE rings (sync + scalar) so accumulation loops can start on
            # chunk 0 instead of waiting for a whole 2MB tensor. Tensor order =
            # compute need order: xk, wk (K), xq, wq (QT), wv (V), wo (outproj).
            rings = [nc.sync, nc.scalar]

            def load_chunked(name, pool, src, cols, first=0):
                a = pool.tile([P, NT, cols], BF, tag=name, name=name)
                src3 = src.rearrange("(t p) c -> p t c", p=P)
                for t in range(first, NT):
                    rings[t % 2].dma_start(a[:, t], src3[:, t])
                return a

            # QT is the biggest dense PE phase: feed it first (xq + wq), then
            # tiny tensors, then xk/wk (K path), wv, wo.
            xq_a = dp.tile([P, NT, QS], BF, tag="xq_a")
            xq_src = xTq.rearrange("(t p) c -> p t c", p=P)
            wq_a = wp.tile([P, NT, D], BF, tag="wq_a")
            wq_src = Wq.rearrange("(t p) c -> p t c", p=P)
            for t in range(4):
                rings[t % 2].dma_start(xq_a[:, t], xq_src[:, t])
                rings[(t + 1) % 2].dma_start(wq_a[:, t], wq_src[:, t])
            bq_a = dp.tile([P, NT], F32, tag="bq_a")
            nc.sync.dma_start(bq_a[:], bq[:])
            bk_a = dp.tile([P, NT], F32, tag="bk_a")
            nc.scalar.dma_start(bk_a[:], bk[:])
            cb_a = dp.tile([KW, H], F32, tag="cb_a")
            nc.sync.dma_start(cb_a[:], cb[:])
            bv_sb = dp.tile([1, D], BF, tag="bvsb")
            nc.scalar.dma_start(bv_sb[:], bv[:])
            bq_t = [bq_a[:, t:t + 1] for t in range(NT)]
            bk_t = [bk_a[:, t:t + 1] for t in range(NT)]
            cb_t = [cb_a[:, h:h + 1] for h in range(H)]
            for t in range(4, NT):
                rings[t % 2].dma_start(xq_a[:, t], xq_src[:, t])
                rings[(t + 1) % 2].dma_start(wq_a[:, t], wq_src[:, t])

            xk_a = dp.tile([P, NT, KW], BF, tag="xk_a")
            nc.sync.dma_start(xk_a[:], xTk.rearrange("(t p) k -> p t k", p=P))
            wk_a = load_chunked("wk_a", wp, Wk, D)
            wv_a = load_chunked("wv_a", wp, Wv, D)
            wo_a = load_chunked("wo_a", wp, Wo, D)
            xk_t = [xk_a[:, t] for t in range(NT)]
            wk_t = [wk_a[:, t] for t in range(NT)]
            wv_t = [wv_a[:, t] for t in range(NT)]
            xq_t = [xq_a[:, t] for t in range(NT)]
            wq_t = [wq_a[:, t] for t in range(NT)]
            wo_t = [wo_a[:, t] for t in range(NT)]

            ones_row = dp.tile([1, P], BF, tag="ones_row")
            nc.vector.memset(ones_row[:], 1.0)
            ones_col = dp.tile([P, 1], BF, tag="ones_col")
            nc.vector.memset(ones_col[:], 1.0)

            # ---- QT[ch, q]: d-outer over ch-tile pairs, paced by wq chunk DMAs ----
            qt_t = []
            for pr in range(NT // 2):
                ps2 = [
                    pacc.tile([P, QS], F32, tag="acc", name=f"qps{pr}_{j}")
                    for j in range(2)
                ]
                for d in range(NT):
                    for j in range(2):
                        t = 2 * pr + j
                        nc.tensor.matmul(
                            ps2[j][:], wq_t[d][:, t * P:(t + 1) * P], xq_t[d][:],
                            start=(d == 0), stop=(d == NT - 1),
                        )
                for j in range(2):
                    t = 2 * pr + j
                    qt = dp.tile([P, QS], BF, tag=f"qt{t}", name=f"qt{t}")
                    nc.vector.tensor_scalar_add(qt[:], ps2[j][:], bq_t[t][:])
                    qt_t.append(qt)

            identity = dp.tile([P, P], BF, tag="identity")
            make_identity(nc, identity[:])

            # ---- K[k, ch] = xk^T Wk (16 big matmuls, d-outer, paced by wk
            # chunks), then 8 PE transposes -> KT[ch, k] + bias ----
            k_sb = dp.tile([P, D], BF, tag="ksb")
            kps = [
                pacc.tile([P, 512], F32, tag="acc", name=f"kps{_b}")
                for _b in range(2)
            ]
            for d in range(NT):
                for blk in range(2):
                    nc.tensor.matmul(
                        kps[blk][:], xk_t[d][:],
                        wk_t[d][:, blk * 512:(blk + 1) * 512],
                        start=(d == 0), stop=(d == NT - 1),
                    )
            for blk in range(2):
                nc.vector.tensor_copy(k_sb[:, blk * 512:(blk + 1) * 512], kps[blk][:])
            kt_t = []
            for t in range(NT):
                tps = psc.tile([P, P], BF, tag="scores", name=f"tps{t}")
                nc.tensor.transpose(tps[:], k_sb[:, t * P:(t + 1) * P], identity[:])
                kt = dp.tile([P, KW], BF, tag=f"kt{t}", name=f"kt{t}")
                nc.vector.tensor_scalar_add(kt[:], tps[:], bk_t[t][:])
                kt_t.append(kt)

            # ---- V[k, ch] (stationary xk_t[d] reused across both 512-blocks) ----
            v_sb = dp.tile([P, D], BF, tag="vsb")
            vps = [pacc.tile([P, 512], F32, tag="acc", name=f"vps{_b}") for _b in range(2)]
            for d in range(NT):
                for blk in range(2):
                    nc.tensor.matmul(
                        vps[blk][:], xk_t[d][:],
                        wv_t[d][:, blk * 512:(blk + 1) * 512],
                        start=(d == 0), stop=False,
                    )
            for blk in range(2):
                nc.tensor.matmul(
                    vps[blk][:], ones_row[:], bv_sb[:, blk * 512:(blk + 1) * 512],
                    start=False, stop=True,
                )
                nc.vector.tensor_copy(v_sb[:, blk * 512:(blk + 1) * 512], vps[blk][:])

            # ---- attention ----
            # Phase A per head: QK -> exp(PT) -> one-hot den-stack matmul; PV pairs.
            # Denominators for heads 0-7 accumulate in den_ps[0], 8-15 in den_ps[1]
            # (rows h%8). One batched reciprocal per stack, then row-scatter DMAs +
            # partition_broadcast, one normalize-multiply per head pair.
            # Per head: QK (row-tiled pairs) -> exp -> denom matmul -> fast
            # reciprocal (fp32, [1,512] psum at p0) -> partition_broadcast
            # (gpsimd DMA; NB offset output base silently writes nothing on HW,
            # so broadcast to a full 128-partition tile) -> normalize-multiply.
            # Each head's chain completes ~3us after its exp, so ot tiles arrive
            # progressively and the out-projection never stalls on the last head.
            pt_t = []
            ot_t = []
            for t in range(NT):
                sps2 = []
                for j in range(2):
                    po = j * 64
                    s_ps = psc.tile([P, QS], F32, tag="scores", name=f"sps{t}_{j}")
                    nc.tensor.matmul(
                        s_ps[:], kt_t[t][po:po + 64, :], qt_t[t][po:po + 64, :],
                        start=True, stop=True,
                    )
                    sps2.append(s_ps)
                for j in range(2):
                    h = 2 * t + j
                    pt = dp.tile([P, QS], BF, tag=f"pt{h % 4}", name=f"pt{h}")
                    nc.scalar.activation(
                        pt[:], sps2[j][:], AF.Exp, bias=cb_t[h][:], scale=SCALE
                    )
                    pt_t.append(pt)
                # PV pair back-to-back: col-tiled (0,0)/(0,64), concurrent.
                pv_ps = patt.tile([P, QS], F32, tag="pv")
                nc.tensor.matmul(
                    pv_ps[0:64, :], v_sb[:, (2 * t) * 64:(2 * t) * 64 + 64],
                    pt_t[2 * t][:], start=True, stop=True, tile_position=(0, 0),
                )
                nc.tensor.matmul(
                    pv_ps[64:128, :], v_sb[:, (2 * t + 1) * 64:(2 * t + 1) * 64 + 64],
                    pt_t[2 * t + 1][:], start=True, stop=True, tile_position=(0, 64),
                )
                pv_sb = fp.tile([P, QS], F32, tag="pvsb", name=f"pvsb{t}")
                nc.vector.tensor_copy(pv_sb[:], pv_ps[:])
                ot = dp.tile([P, QS], BF, tag=f"ot{t}", name=f"ot{t}")
                for j in range(2):
                    h = 2 * t + j
                    po = j * 64
                    d_ps = psml.tile([1, QS], F32, tag="den", name=f"dps{h}")
                    nc.tensor.matmul(
                        d_ps[:], ones_col[:], pt_t[h][:], start=True, stop=True
                    )
                    r0 = fp.tile([1, QS], F32, tag=f"rcp0_{h % 4}", name=f"r0_{h}")
                    nc.vector.reciprocal_approx_fast(r0[:], d_ps[:])
                    rc_bc = fp.tile([P, QS], F32, tag="rcbc", name=f"rcbc{h}")
                    nc.gpsimd.partition_broadcast(rc_bc[:], r0[:], channels=P)
                    nc.vector.tensor_mul(
                        ot[po:po + 64, :], pv_sb[po:po + 64, :],
                        rc_bc[po:po + 64, :],
                    )
                ot_t.append(ot)

            # ---- output projection out[q, d] = outT^T Wo
            # (stationary ot slice reused across both 512-blocks) ----
            for qi in range(QS // P):
                ops = [pacc.tile([P, 512], F32, tag="acc", name=f"ops{qi}_{_b}") for _b in range(2)]
                for tt in range(NT):
                    for blk in range(2):
                        nc.tensor.matmul(
                            ops[blk][:], ot_t[tt][:, qi * P:(qi + 1) * P],
                            wo_t[tt][:, blk * 512:(blk + 1) * 512],
                            start=(tt == 0), stop=(tt == NT - 1),
                        )
                o_sb = fp.tile([P, 2, 512], F32, tag="osb")
                for blk in range(2):
                    nc.vector.tensor_copy(o_sb[:, blk], ops[blk][:])
                    rings[blk].dma_start(
                        out[qi * P:(qi + 1) * P, blk * 512:(blk + 1) * 512],
                        o_sb[:, blk],
                    )

    nc.compile()
    return nc


def _get_nc():
    if "nc" not in _CACHE:
        _CACHE["nc"] = _build()
    return _CACHE["nc"]


def _in_maps(x, Wq, bq, Wk, bk, Wv, bv, Wo, bo):
    bf = ml_dtypes.bfloat16
    f32 = np.float32
    x = np.asarray(x, f32)
    xT = np.ascontiguousarray(np.transpose(x, (0, 2, 1)))  # [B, D, S]
    wq = np.asarray(Wq, f32).astype(bf)
    wk = np.asarray(Wk, f32).astype(bf)
    wv = np.asarray(Wv, f32).astype(bf)
    wo = np.asarray(Wo, f32).astype(bf)
    bq2 = np.ascontiguousarray(np.asarray(bq, f32).reshape(NT, P).T)
    bk2 = np.ascontiguousarray(np.asarray(bk, f32).reshape(NT, P).T)
    bv2 = np.asarray(bv, f32).astype(bf).reshape(1, D)
    slopes = 1.0 / 2.0 ** (np.arange(H, dtype=np.float64) / H)
    ks = np.arange(K0, S, dtype=np.float64)
    cbias = np.ascontiguousarray(
        (slopes[:, None] * (ks[None, :] - (S - 1))).astype(f32).T
    )
    maps = []
    for c in range(N_CORES):
        b, q0 = c // 4, (c % 4) * QS
        maps.append({
            "xTq": np.ascontiguousarray(xT[b, :, q0:q0 + QS]).astype(bf),
            "xTk": np.ascontiguousarray(xT[b, :, K0:S]).astype(bf),
            "Wq": wq, "Wk": wk, "Wv": wv, "Wo": wo,
            "bq": bq2, "bk": bk2, "bv": bv2, "cbias": cbias,
        })
    return maps


def _run(inputs, trace=False, tmpdir=None):
    from concourse.bass_utils import run_bass_kernel_spmd

    nc = _get_nc()
    maps = _in_maps(**inputs)
    res = run_bass_kernel_spmd(
        nc, maps, core_ids=list(range(N_CORES)), trace=trace, tmpdir=tmpdir
    )
    bo = np.asarray(inputs["bo"], np.float32)
    full = np.zeros((B, S, D), np.float32)
    for c in range(N_CORES):
        b, q0 = c // 4, (c % 4) * QS
        full[b, q0:q0 + QS] = res.results[c]["out"]
    full += bo[None, None, :]
    return full, res


def kernel(**inputs) -> np.ndarray:
    return _run(inputs, trace=False)[0]


# revision 39
# speedup vs baseline: 1.6607x; 1.6607x over previous
"""ALiBi attention (B=2, S=2048, D=1024, H=16) on 8 TRN2 NeuronCores.

Sharding: core c handles batch b = c//4 and query slice q0 = (c%4)*512. No
collectives -- cores are fully independent; the host concatenates q-slices.

Key insight (the "sparse" in sparse_attention): the reference applies bias
slope_h*(k-q) with NO causal mask and slopes in [0.52, 1.0], so softmax mass
sits entirely on the last ~60 keys. Keeping only the last KW=128 keys gives a
max attention-weight error ~1e-29. Further, with rowmax ~= slope*(S-1-q),
exp(qk*scale + slope*(k-q) - rowmax) == exp(qk*scale + slope*(k-S+1)): the
shift is purely a function of k, i.e. a per-partition constant in the [k, q]
layout -> the whole softmax numerator is ONE fused ACT op per head
(exp(scale*in + cbias_h) reading the QK^T PSUM directly).

Per core (all matmuls bf16 with fp32 PSUM accumulation):
  QT[ch,q]  = Wq^T xq^T    64 mm N=512, d-accumulated, paced by wq chunk DMAs
  K[k,ch]   = xk^T Wk      16 mm N=512, then 8 PE transposes -> KT[ch,k]
  V[k,ch]   = xk Wv        16 mm N=512 (+bias via K=1 ones matmul)
  scoresT_h = KT_h^T QT_h  [k=128, q=512], one mm per head (row-tiled pairs)
  PT_h      = exp(scale*scoresT_h + cbias_h)   one ACT op, PSUM -> SBUF bf16
  den pair  = onehot^T PT  two accumulating mms -> [2, 512] PSUM, then ONE
              batched reciprocal_approx_fast, gpsimd partition_broadcast to
              full tiles (offset-base broadcast silently no-ops on HW!),
              normalize-multiply straight out of the PV PSUM
  outT_h    = V_h^T PT_h   col-tiled pairs into one [128, 512] PSUM
  out[q,d]  = outT^T Wo    64 mm N=512 + bf16 out DMA (+ bo on host)

Perf structure: weights stream as per-d-tile 256KB chunks alternating across
the two HWDGE rings (sync+scalar) in compute-need order (xq/wq interleaved
first, then xk/wk, wv, wo); QT runs d-outer so pass 0 is paced by arrivals;
PSUM banks are time-shared across phases by alternating pool tags
(QT: acc/scores, outproj: acc/pv); per-pair denominator chains keep the
normalization off the PE critical path. HW exec ~75-90us/core (varies with
process/HAM phase), rel err ~4e-3 vs the fp32 reference.
"""

import numpy as np
import ml_dtypes

D = 1024
H = 16
HD = 64
B = 2
S = 2048
QS = 512          # queries per core
KW = 128          # key window (last KW keys carry all softmax mass)
K0 = S - KW
NT = 8            # 128-wide tiles over D
P = 128
SCALE = HD ** -0.5
N_CORES = 8

_CACHE = {}


PARAMS = {
    "acc_bufs": 3, "scores_bufs": 2, "pv_bufs": 2, "psml_bufs": 1,
    "fp_bufs": 3, "pt_ring": 4, "pv_copy": False, "wvwo_interleave": False,
    "k_first": False, "pair_den": True, "tag_alt": True, "crit_pairs": False,
    "gps_dma": False,
}


def _build(params=None):
    p_ = dict(PARAMS)
    if params:
        p_.update(params)
    import concourse.bacc as bacc
    import concourse.mybir as mybir
    import concourse.tile as tile
    from concourse.masks import make_identity

    BF = mybir.dt.bfloat16
    F32 = mybir.dt.float32
    AF = mybir.ActivationFunctionType

    nc = bacc.Bacc("TRN2", target_bir_lowering=False, debug=False, num_devices=N_CORES)

    xTq = nc.dram_tensor("xTq", [D, QS], BF, kind="ExternalInput").ap()
    xTk = nc.dram_tensor("xTk", [D, KW], BF, kind="ExternalInput").ap()
    Wq = nc.dram_tensor("Wq", [D, D], BF, kind="ExternalInput").ap()
    Wk = nc.dram_tensor("Wk", [D, D], BF, kind="ExternalInput").ap()
    Wv = nc.dram_tensor("Wv", [D, D], BF, kind="ExternalInput").ap()
    Wo = nc.dram_tensor("Wo", [D, D], BF, kind="ExternalInput").ap()
    bq = nc.dram_tensor("bq", [P, NT], F32, kind="ExternalInput").ap()
    bk = nc.dram_tensor("bk", [P, NT], F32, kind="ExternalInput").ap()
    bv = nc.dram_tensor("bv", [1, D], BF, kind="ExternalInput").ap()
    cb = nc.dram_tensor("cbias", [KW, H], F32, kind="ExternalInput").ap()
    out = nc.dram_tensor("out", [QS, D], BF, kind="ExternalOutput").ap()

    with tile.TileContext(nc) as tc:
        with (
            tc.tile_pool(name="wpool", bufs=1) as wp,
            tc.tile_pool(name="dpool", bufs=1) as dp,
            tc.tile_pool(name="flow", bufs=p_["fp_bufs"]) as fp,
            tc.tile_pool(name="pacc", bufs=p_["acc_bufs"], space="PSUM") as pacc,
            tc.tile_pool(name="patt", bufs=p_["pv_bufs"], space="PSUM") as patt,
            tc.tile_pool(name="psc", bufs=p_["scores_bufs"], space="PSUM") as psc,
            tc.tile_pool(name="psml", bufs=p_["psml_bufs"], space="PSUM") as psml,
        ):
            # ---- load inputs. Per-d-tile chunks (256KB) alternating across the
            # two HWDGE rings (sync + scalar) so accumulation loops can start on
            # chunk 0 instead of waiting for a whole 2MB tensor. Tensor order =
            # compute need order: xk, wk (K), xq, wq (QT), wv (V), wo (outproj).
            rings = [nc.sync, nc.scalar]

            def load_chunked(name, pool, src, cols, first=0):
                a = pool.tile([P, NT, cols], BF, tag=name, name=name)
                src3 = src.rearrange("(t p) c -> p t c", p=P)
                for t in range(first, NT):
                    rings[t % 2].dma_start(a[:, t], src3[:, t])
                return a

            # QT is the biggest dense PE phase: feed it first (xq + wq), then
            # tiny tensors, then xk/wk (K path), wv, wo.
            if p_["k_first"]:
                xk_a0 = dp.tile([P, NT, KW], BF, tag="xk_a", name="xk_a")
                nc.sync.dma_start(xk_a0[:], xTk.rearrange("(t p) k -> p t k", p=P))
                wk_a0 = load_chunked("wk_a", wp, Wk, D)
            xq_a = dp.tile([P, NT, QS], BF, tag="xq_a")
            xq_src = xTq.rearrange("(t p) c -> p t c", p=P)
            wq_a = wp.tile([P, NT, D], BF, tag="wq_a")
            wq_src = Wq.rearrange("(t p) c -> p t c", p=P)
            for t in range(4):
                rings[t % 2].dma_start(xq_a[:, t], xq_src[:, t])
                rings[(t + 1) % 2].dma_start(wq_a[:, t], wq_src[:, t])
            small_eng = nc.gpsimd if p_["gps_dma"] else nc.sync
            small_eng2 = nc.gpsimd if p_["gps_dma"] else nc.scalar
            bq_a = dp.tile([P, NT], F32, tag="bq_a")
            small_eng.dma_start(bq_a[:], bq[:])
            bk_a = dp.tile([P, NT], F32, tag="bk_a")
            small_eng2.dma_start(bk_a[:], bk[:])
            cb_a = dp.tile([KW, H], F32, tag="cb_a")
            small_eng.dma_start(cb_a[:], cb[:])
            bv_sb = dp.tile([1, D], BF, tag="bvsb")
            small_eng2.dma_start(bv_sb[:], bv[:])
            bq_t = [bq_a[:, t:t + 1] for t in range(NT)]
            bk_t = [bk_a[:, t:t + 1] for t in range(NT)]
            cb_t = [cb_a[:, h:h + 1] for h in range(H)]
            for t in range(4, NT):
                rings[t % 2].dma_start(xq_a[:, t], xq_src[:, t])
                rings[(t + 1) % 2].dma_start(wq_a[:, t], wq_src[:, t])

            if p_["k_first"]:
                xk_a = xk_a0
                wk_a = wk_a0
            else:
                xk_a = dp.tile([P, NT, KW], BF, tag="xk_a", name="xk_a")
                nc.sync.dma_start(xk_a[:], xTk.rearrange("(t p) k -> p t k", p=P))
                wk_a = load_chunked("wk_a", wp, Wk, D)
            if p_["wvwo_interleave"]:
                wv_a = wp.tile([P, NT, D], BF, tag="wv_a", name="wv_a")
                wo_a = wp.tile([P, NT, D], BF, tag="wo_a", name="wo_a")
                wv_src = Wv.rearrange("(t p) c -> p t c", p=P)
                wo_src = Wo.rearrange("(t p) c -> p t c", p=P)
                for t in range(NT):
                    rings[t % 2].dma_start(wv_a[:, t], wv_src[:, t])
                    rings[(t + 1) % 2].dma_start(wo_a[:, t], wo_src[:, t])
            else:
                wv_a = load_chunked("wv_a", wp, Wv, D)
                wo_a = load_chunked("wo_a", wp, Wo, D)
            xk_t = [xk_a[:, t] for t in range(NT)]
            wk_t = [wk_a[:, t] for t in range(NT)]
            wv_t = [wv_a[:, t] for t in range(NT)]
            xq_t = [xq_a[:, t] for t in range(NT)]
            wq_t = [wq_a[:, t] for t in range(NT)]
            wo_t = [wo_a[:, t] for t in range(NT)]

            ones_row = dp.tile([1, P], BF, tag="ones_row")
            nc.vector.memset(ones_row[:], 1.0)
            ones_col = dp.tile([P, 1], BF, tag="ones_col")
            nc.vector.memset(ones_col[:], 1.0)
            oh2 = dp.tile([P, 4], BF, tag="oh2")
            nc.vector.memset(oh2[:], 0.0)
            nc.vector.memset(oh2[:, 0:1], 1.0)
            nc.vector.memset(oh2[:, 3:4], 1.0)
            ones_r64 = dp.tile([1, 64], F32, tag="ones_r64")
            nc.vector.memset(ones_r64[:], 1.0)

            # ---- QT[ch, q]: d-outer over ch-tile pairs, paced by wq chunk DMAs ----
            qt_t = []
            for t in range(NT):
                if p_["tag_alt"] and t % 2 == 1:
                    ps = psc.tile([P, QS], F32, tag="scores", name=f"qps{t}")
                else:
                    ps = pacc.tile([P, QS], F32, tag="acc", name=f"qps{t}")
                for d in range(NT):
                    nc.tensor.matmul(
                        ps[:], wq_t[d][:, t * P:(t + 1) * P], xq_t[d][:],
                        start=(d == 0), stop=(d == NT - 1),
                    )
                qt = dp.tile([P, QS], BF, tag=f"qt{t}", name=f"qt{t}")
                nc.vector.tensor_scalar_add(qt[:], ps[:], bq_t[t][:])
                qt_t.append(qt)

            identity = dp.tile([P, P], BF, tag="identity")
            make_identity(nc, identity[:])

            # ---- K[k, ch] = xk^T Wk (16 big matmuls, d-outer, paced by wk
            # chunks), then 8 PE transposes -> KT[ch, k] + bias ----
            k_sb = dp.tile([P, D], BF, tag="ksb")
            kps = [
                pacc.tile([P, 512], F32, tag="acc", name=f"kps{_b}")
                for _b in range(2)
            ]
            for d in range(NT):
                for blk in range(2):
                    nc.tensor.matmul(
                        kps[blk][:], xk_t[d][:],
                        wk_t[d][:, blk * 512:(blk + 1) * 512],
                        start=(d == 0), stop=(d == NT - 1),
                    )
            for blk in range(2):
                nc.vector.tensor_copy(k_sb[:, blk * 512:(blk + 1) * 512], kps[blk][:])
            kt_t = []
            for t in range(NT):
                tps = psc.tile([P, P], BF, tag="scores", name=f"tps{t}")
                nc.tensor.transpose(tps[:], k_sb[:, t * P:(t + 1) * P], identity[:])
                kt = dp.tile([P, KW], BF, tag=f"kt{t}", name=f"kt{t}")
                nc.vector.tensor_scalar_add(kt[:], tps[:], bk_t[t][:])
                kt_t.append(kt)

            # ---- V[k, ch] (stationary xk_t[d] reused across both 512-blocks) ----
            v_sb = dp.tile([P, D], BF, tag="vsb")
            vps = [pacc.tile([P, 512], F32, tag="acc", name=f"vps{_b}") for _b in range(2)]
            for d in range(NT):
                for blk in range(2):
                    nc.tensor.matmul(
                        vps[blk][:], xk_t[d][:],
                        wv_t[d][:, blk * 512:(blk + 1) * 512],
                        start=(d == 0), stop=False,
                    )
            for blk in range(2):
                nc.tensor.matmul(
                    vps[blk][:], ones_row[:], bv_sb[:, blk * 512:(blk + 1) * 512],
                    start=False, stop=True,
                )
                nc.vector.tensor_copy(v_sb[:, blk * 512:(blk + 1) * 512], vps[blk][:])

            # ---- attention ----
            # Phase A per head: QK -> exp(PT) -> one-hot den-stack matmul; PV pairs.
            # Denominators for heads 0-7 accumulate in den_ps[0], 8-15 in den_ps[1]
            # (rows h%8). One batched reciprocal per stack, then row-scatter DMAs +
            # partition_broadcast, one normalize-multiply per head pair.
            # Per head: QK (row-tiled pairs) -> exp -> denom matmul -> fast
            # reciprocal (fp32, [1,512] psum at p0) -> partition_broadcast
            # (gpsimd DMA; NB offset output base silently writes nothing on HW,
            # so broadcast to a full 128-partition tile) -> normalize-multiply.
            # Each head's chain completes ~3us after its exp, so ot tiles arrive
            # progressively and the out-projection never stalls on the last head.
            pt_t = []
            ot_t = []
            for t in range(NT):
                sps2 = []
                import contextlib
                crit = tc.tile_critical if p_["crit_pairs"] else contextlib.nullcontext
                with crit():
                    for j in range(2):
                        po = j * 64
                        s_ps = psc.tile(
                            [P, QS], F32, tag="scores", name=f"sps{t}_{j}"
                        )
                        nc.tensor.matmul(
                            s_ps[:], kt_t[t][po:po + 64, :],
                            qt_t[t][po:po + 64, :], start=True, stop=True,
                        )
                        sps2.append(s_ps)
                for j in range(2):
                    h = 2 * t + j
                    pt = dp.tile(
                        [P, QS], BF, tag=f"pt{h % p_['pt_ring']}", name=f"pt{h}"
                    )
                    nc.scalar.activation(
                        pt[:], sps2[j][:], AF.Exp, bias=cb_t[h][:], scale=SCALE
                    )
                    pt_t.append(pt)
                # PV pair back-to-back: col-tiled (0,0)/(0,64), concurrent.
                pv_ps = patt.tile([P, QS], F32, tag="pv")
                with crit():
                    nc.tensor.matmul(
                        pv_ps[0:64, :], v_sb[:, (2 * t) * 64:(2 * t) * 64 + 64],
                        pt_t[2 * t][:], start=True, stop=True,
                        tile_position=(0, 0),
                    )
                    nc.tensor.matmul(
                        pv_ps[64:128, :],
                        v_sb[:, (2 * t + 1) * 64:(2 * t + 1) * 64 + 64],
                        pt_t[2 * t + 1][:], start=True, stop=True,
                        tile_position=(0, 64),
                    )
                if p_["pv_copy"]:
                    pv_sb = fp.tile([P, QS], F32, tag="pvsb", name=f"pvsb{t}")
                    nc.vector.tensor_copy(pv_sb[:], pv_ps[:])
                else:
                    pv_sb = pv_ps
                ot = dp.tile([P, QS], BF, tag=f"ot{t}", name=f"ot{t}")
                if p_["pair_den"]:
                    # both heads' denoms into one [2, QS] psum (one-hot cols),
                    # single batched reciprocal, row 1 rescattered to p0.
                    d_ps = psml.tile([2, QS], F32, tag="den", name=f"dps{t}")
                    nc.tensor.matmul(
                        d_ps[:], oh2[:, 0:2], pt_t[2 * t][:],
                        start=True, stop=False,
                    )
                    nc.tensor.matmul(
                        d_ps[:], oh2[:, 2:4], pt_t[2 * t + 1][:],
                        start=False, stop=True,
                    )
                    rr = fp.tile([2, QS], F32, tag="rr", name=f"rr{t}")
                    nc.vector.reciprocal_approx_fast(rr[:], d_ps[:])
                    r1 = fp.tile([1, QS], F32, tag="r1", name=f"r1_{t}")
                    (nc.gpsimd if p_["gps_dma"] else nc.sync).dma_start(
                        r1[:], rr[1:2, :]
                    )
                    rsrc = [rr[0:1, :], r1[:]]
                    for j in range(2):
                        h = 2 * t + j
                        po = j * 64
                        rc_bc = fp.tile([P, QS], F32, tag="rcbc", name=f"rcbc{h}")
                        nc.gpsimd.partition_broadcast(
                            rc_bc[:], rsrc[j], channels=P
                        )
                        nc.vector.tensor_mul(
                            ot[po:po + 64, :], pv_sb[po:po + 64, :],
                            rc_bc[po:po + 64, :],
                        )
                else:
                    for j in range(2):
                        h = 2 * t + j
                        po = j * 64
                        d_ps = psml.tile([1, QS], F32, tag="den", name=f"dps{h}")
                        nc.tensor.matmul(
                            d_ps[:], ones_col[:], pt_t[h][:], start=True, stop=True
                        )
                        r0 = fp.tile(
                            [1, QS], F32, tag=f"rcp0_{h % 4}", name=f"r0_{h}"
                        )
                        nc.vector.reciprocal_approx_fast(r0[:], d_ps[:])
                        rc_bc = fp.tile([P, QS], F32, tag="rcbc", name=f"rcbc{h}")
                        nc.gpsimd.partition_broadcast(rc_bc[:], r0[:], channels=P)
                        nc.vector.tensor_mul(
                            ot[po:po + 64, :], pv_sb[po:po + 64, :],
                            rc_bc[po:po + 64, :],
                        )
                ot_t.append(ot)

            # ---- output projection out[q, d] = outT^T Wo
            # (stationary ot slice reused across both 512-blocks) ----
            for qi in range(QS // P):
                for blk in range(2):
                    if p_["tag_alt"] and blk == 1:
                        ops = patt.tile(
                            [P, 512], F32, tag="pv", name=f"ops{qi}_{blk}"
                        )
                    else:
                        ops = pacc.tile(
                            [P, 512], F32, tag="acc", name=f"ops{qi}_{blk}"
                        )
                    for tt in range(NT):
                        nc.tensor.matmul(
                            ops[:], ot_t[tt][:, qi * P:(qi + 1) * P],
                            wo_t[tt][:, blk * 512:(blk + 1) * 512],
                            start=(tt == 0), stop=(tt == NT - 1),
                        )
                    o_sb = fp.tile([P, 512], BF, tag="osb", name=f"osb{qi}_{blk}")
                    nc.vector.tensor_copy(o_sb[:], ops[:])
                    oeng = nc.gpsimd if p_["gps_dma"] else rings[blk]
                    oeng.dma_start(
                        out[qi * P:(qi + 1) * P, blk * 512:(blk + 1) * 512],
                        o_sb[:],
                    )

    nc.compile()
    return nc


def _get_nc():
    if "nc" not in _CACHE:
        _CACHE["nc"] = _build()
    return _CACHE["nc"]


def _in_maps(x, Wq, bq, Wk, bk, Wv, bv, Wo, bo):
    bf = ml_dtypes.bfloat16
    f32 = np.float32
    x = np.asarray(x, f32)
    xT = np.ascontiguousarray(np.transpose(x, (0, 2, 1)))  # [B, D, S]
    wq = np.asarray(Wq, f32).astype(bf)
    wk = np.asarray(Wk, f32).astype(bf)
    wv = np.asarray(Wv, f32).astype(bf)
    wo = np.asarray(Wo, f32).astype(bf)
    bq2 = np.ascontiguousarray(np.asarray(bq, f32).reshape(NT, P).T)
    bk2 = np.ascontiguousarray(np.asarray(bk, f32).reshape(NT, P).T)
    bv2 = np.asarray(bv, f32).astype(bf).reshape(1, D)
    slopes = 1.0 / 2.0 ** (np.arange(H, dtype=np.float64) / H)
    ks = np.arange(K0, S, dtype=np.float64)
    cbias = np.ascontiguousarray(
        (slopes[:, None] * (ks[None, :] - (S - 1))).astype(f32).T
    )
    maps = []
    for c in range(N_CORES):
        b, q0 = c // 4, (c % 4) * QS
        maps.append({
            "xTq": np.ascontiguousarray(xT[b, :, q0:q0 + QS]).astype(bf),
            "xTk": np.ascontiguousarray(xT[b, :, K0:S]).astype(bf),
            "Wq": wq, "Wk": wk, "Wv": wv, "Wo": wo,
            "bq": bq2, "bk": bk2, "bv": bv2, "cbias": cbias,
        })
    return maps


def _run(inputs, trace=False, tmpdir=None):
    from concourse.bass_utils import run_bass_kernel_spmd

    nc = _get_nc()
    maps = _in_maps(**inputs)
    try:
        res = run_bass_kernel_spmd(
            nc, maps, core_ids=list(range(N_CORES)), trace=trace, tmpdir=tmpdir
        )
    except Exception:
        # transient device faults (NRT_EXEC_UNIT_UNRECOVERABLE) happen rarely;
        # one retry on a fresh attempt clears them
        res = run_bass_kernel_spmd(
            nc, maps, core_ids=list(range(N_CORES)), trace=trace, tmpdir=tmpdir
        )
    bo = np.asarray(inputs["bo"], np.float32)
    full = np.zeros((B, S, D), np.float32)
    for c in range(N_CORES):
        b, q0 = c // 4, (c % 4) * QS
        full[b, q0:q0 + QS] = res.results[c]["out"].astype(np.float32)
    full += bo[None, None, :]
    return full, res


def kernel(**inputs) -> np.ndarray:
    return _run(inputs, trace=False)[0]
